# revision 34
# baseline (speedup 1.0000x reference)
"""Trainium2 Bass kernel for nn_NodeNet (GNN message passing).

Strategy: data-parallel over graphs across 8 NeuronCores. Host transposes
inputs into [feature, row] layouts so every DMA is contiguous; all matmuls
run in bf16 with transposed activations:
  node stage: dp[128, rows] -> MLP -> sum over datapoints -> feature_enc[64, G]
  edge stage: rhs = [fe (broadcast per graph); edge_attr^T] -> MLP -> out^T

Structure: 512-column matmul slices, grouped in pairs (1024 cols) and
super-pairs (2048 cols). Within a super-pair matmuls are emitted
weight-major so each stationary operand loads once (LDWEIGHTS switches
halved). PSUM: L1 uses per-slice single-bank tiles (low-latency drains,
3-deep rotation); L2 uses pair-merged [128, 2, 512] two-bank tiles whose
halves hold the same feature chunk of both slices, so one ScalarE/VectorE
op drains 1024 columns with a single per-partition bias; L3 outputs of a
pair share one bank via column tiling (partitions 0-63 / 64-127) with a
single k-outer accumulation group. Drains alternate ScalarE/VectorE;
the per-graph feature_enc broadcast rotates gpsimd/DVE/ACT. Edge columns
are host-permuted into (rep, graph) interleaved order so the broadcast
has a contiguous innermost axis; host un-permutes the output. Outputs
store as bf16 and are upcast on host.

The structured fast path (edges grouped 128-per-graph, all within-graph,
as produced by the reference's setup_inputs) runs one fused launch per
core with feature_enc kept on-chip. A general fallback handles arbitrary
edge_index / batch with two launches and a host-side gather + mask.
"""

import os
import sys

import ml_dtypes
import numpy as np

BF16NP = ml_dtypes.bfloat16

if "/opt/trn_rl_repo" not in sys.path and os.path.isdir("/opt/trn_rl_repo"):
    sys.path.insert(0, "/opt/trn_rl_repo")

import concourse.bacc as bacc
import concourse.tile as tile
from concourse import mybir
from concourse.bass_utils import run_bass_kernel_spmd

G, ODE, NDATA, H, EA, EPG = 4096, 64, 32, 256, 64, 128
E = G * EPG
NCORES = 8
GC = G // NCORES           # graphs per core
RC = GC * NDATA            # node-MLP rows per core
EC = GC * EPG              # edges per core
TN = 512                   # matmul slice free size (one PSUM bank of fp32)
PN = 2 * TN                # pair free size
GT = TN // NDATA           # graphs covered per node slice (16)
GEP = PN // EPG            # graphs covered per edge pair (8)

F32 = mybir.dt.float32
BF16 = mybir.dt.bfloat16
RELU = mybir.ActivationFunctionType.Relu
IDENT = mybir.ActivationFunctionType.Identity
ADD = mybir.AluOpType.add
MAX = mybir.AluOpType.max
BYPASS = mybir.AluOpType.bypass
AXX = mybir.AxisListType.X

FEBCAST = os.environ.get("FEBCAST", "va")  # va | rot | gpsimd | ve | act
NODE_RED = os.environ.get("NODE_RED", "gp")  # gp (gpsimd TT halving) | ve

_PROGRAMS = {}
last_results = None


def _edge_col_perm():
    """Column -> edge mapping for the interleaved edge ordering.

    Within each pair p of matmul slices (PN=1024 columns, GEP=8 graphs),
    column j holds edge (rep r = j // GEP) of graph (g = j % GEP), i.e.
    local edge (8p + g) * EPG + r.
    """
    col = np.arange(EC)
    p, j = col // PN, col % PN
    r, g = j // GEP, j % GEP
    return (GEP * p + g) * EPG + r


_ECOL = _edge_col_perm()


def _install_trace_shim():
    """Optional: make trace=True work by injecting antenv.axon_hooks."""
    import types

    if "antenv.axon_hooks" in sys.modules:
        return
    try:
        mod = types.ModuleType("antenv.axon_hooks")
        mod._hook = None
        mod.set_axon_ntff_profile_hook = lambda h: setattr(mod, "_hook", h)
        mod.get_axon_ntff_profile_hook = lambda: mod._hook
        sys.modules["antenv.axon_hooks"] = mod
        import antenv

        antenv.axon_hooks = mod
        from trn_agent_boot.trn_boot import _ntff_profile_via_ctypes

        hook = _ntff_profile_via_ctypes("/opt/axon/libaxon_pjrt.so")
        if hook is not None:
            mod.set_axon_ntff_profile_hook(hook)
    except Exception:
        pass


def _declare_weights(nc):
    t = {}
    t["nw1"] = nc.dram_tensor("nw1", [128, H], BF16, kind="ExternalInput")
    t["nw2"] = nc.dram_tensor("nw2", [128, 2, H], BF16, kind="ExternalInput")
    t["nw3"] = nc.dram_tensor("nw3", [128, 2, ODE], BF16, kind="ExternalInput")
    t["nb1"] = nc.dram_tensor("nb1", [128, 2], F32, kind="ExternalInput")
    t["nb2"] = nc.dram_tensor("nb2", [128, 2], F32, kind="ExternalInput")
    t["nb3"] = nc.dram_tensor("nb3", [ODE, 1], F32, kind="ExternalInput")
    t["ew1"] = nc.dram_tensor("ew1", [128, H], BF16, kind="ExternalInput")
    t["ew2"] = nc.dram_tensor("ew2", [128, 2, H], BF16, kind="ExternalInput")
    t["ew3"] = nc.dram_tensor("ew3", [128, 2, ODE], BF16, kind="ExternalInput")
    t["eb1"] = nc.dram_tensor("eb1", [128, 2], F32, kind="ExternalInput")
    t["eb2"] = nc.dram_tensor("eb2", [128, 2], F32, kind="ExternalInput")
    t["eb3x"] = nc.dram_tensor("eb3x", [128, 1], F32, kind="ExternalInput")
    return t


def _load_weights(nc, consts, td, node: bool, edge: bool):
    sb = {}
    names = []
    if node:
        names += ["nw1", "nw2", "nw3", "nb1", "nb2", "nb3"]
    if edge:
        names += ["ew1", "ew2", "ew3", "eb1", "eb2", "eb3x"]
    for n in names:
        d = td[n]
        sb[n] = consts.tile(list(d.shape), d.dtype, tag=n, name=n)
        nc.sync.dma_start(sb[n], d[:])
    return sb


def _drain(nc, engine, out, psum, bias, relu=True):
    """PSUM -> SBUF with bias add (+ optional relu) on the chosen engine."""
    if engine == "A":
        nc.scalar.activation(out, psum, RELU if relu else IDENT, bias=bias)
    else:
        nc.vector.tensor_scalar(
            out=out, in0=psum, scalar1=bias, scalar2=0.0,
            op0=ADD, op1=MAX if relu else BYPASS,
        )


def _bcast_fe(nc, p, rt, feT_sb):
    """Fill rt[0:64] with feature_enc of this pair's GEP graphs, repeated
    per edge (interleaved ordering: innermost axis is the GEP graphs)."""
    bc_out = rt[0:64].rearrange("c t (r g) -> c (t r) g", g=GEP)
    bc_in = feT_sb[:, None, p * GEP:(p + 1) * GEP].to_broadcast(
        [ODE, PN // GEP, GEP])
    mode = FEBCAST
    if mode == "rot":
        mode = ("gpsimd", "ve", "act")[p % 3]
    elif mode == "va":
        mode = ("ve", "act")[p % 2]
    if mode == "gpsimd":
        nc.gpsimd.tensor_copy(out=bc_out, in_=bc_in)
    elif mode == "act":
        nc.scalar.copy(bc_out, bc_in)
    else:
        nc.vector.tensor_copy(out=bc_out, in_=bc_in)


def _super_pair(nc, pools, rts, w1, w2, b1, b2, h1_engines, h2_engines):
    """Layers 1+2 for a super-pair (two pairs), emitted weight-major.

    rts: two rhs tiles [128, 2, TN]. Returns two h2 tiles
    [128, 2(chunk), 2(slice), TN] in bf16."""
    consts, xin, hid, oot, psA, psB = pools
    l1 = {}
    for c in (0, 1):
        for i in (0, 1):
            for t in (0, 1):
                l1[(i, c, t)] = psA.tile([128, TN], F32, tag="l1",
                                         name=f"l1_{i}{c}{t}")
    # weight-major L1: one LDWEIGHTS per chunk for all four slices
    for c in (0, 1):
        for i in (0, 1):
            for t in (0, 1):
                nc.tensor.matmul(l1[(i, c, t)], w1[:, 128 * c:128 * (c + 1)],
                                 rts[i][:, t], start=True, stop=True)
    h1s = [hid.tile([128, 2, 2, TN], BF16, tag="h1", name=f"h1_{i}")
           for i in (0, 1)]
    for c in (0, 1):
        for i in (0, 1):
            for t in (0, 1):
                _drain(nc, h1_engines[(i, c, t)], h1s[i][:, c, t], l1[(i, c, t)],
                       b1[:, c:c + 1])
    l2 = {}
    for c in (0, 1):
        for i in (0, 1):
            l2[(i, c)] = psB.tile([128, 2, TN], F32, tag="l2",
                                  name=f"l2_{i}{c}")
    # weight-major L2: one LDWEIGHTS per (k, chunk) for all four slices
    for c in (0, 1):
        for k in (0, 1):
            for i in (0, 1):
                for t in (0, 1):
                    nc.tensor.matmul(l2[(i, c)][:, t], w2[:, k, 128 * c:128 * (c + 1)],
                                     h1s[i][:, k, t], start=(k == 0), stop=(k == 1))
    h2s = [hid.tile([128, 2, 2, TN], BF16, tag="h2", name=f"h2_{i}")
           for i in (0, 1)]
    for c in (0, 1):
        for i in (0, 1):
            _drain(nc, h2_engines[(i, c)], h2s[i][:, c], l2[(i, c)], b2[:, c:c + 1])
    return h2s


NODE_H1E = {(0, 0, 0): "A", (0, 0, 1): "A", (0, 1, 0): "V", (0, 1, 1): "A",
            (1, 0, 0): "A", (1, 0, 1): "A", (1, 1, 0): "V", (1, 1, 1): "A"}
EDGE_H1E = {(0, 0, 0): "A", (0, 0, 1): "A", (0, 1, 0): "V", (0, 1, 1): "V",
            (1, 0, 0): "A", (1, 0, 1): "A", (1, 1, 0): "V", (1, 1, 1): "V"}
H2E = {(0, 0): "A", (0, 1): "V", (1, 0): "A", (1, 1): "V"}


def _node_sp(nc, pools, w, xT_d, hsum, q):
    """One node super-pair: rows [q*2*PN, (q+1)*2*PN), 64 graphs. Writes
    hsum[:, :, q*64:(q+1)*64]."""
    consts, xin, hid, oot, psA, psB = pools
    rts = []
    for i in (0, 1):
        p = 2 * q + i
        xt = xin.tile([128, 2, TN], BF16, tag="xt")
        nc.sync.dma_start(
            xt, xT_d[:, p * PN:(p + 1) * PN].rearrange("c (t e) -> c t e", t=2))
        rts.append(xt)
    h2s = _super_pair(nc, pools, rts, w["nw1"], w["nw2"], w["nb1"], w["nb2"],
                      NODE_H1E, H2E)
    for i in (0, 1):
        p = 2 * q + i
        h2r = h2s[i].rearrange("c k t (g d) -> c (k t g) d", d=NDATA)
        o0 = p * 2 * GT
        with nc.allow_low_precision(reason="bf16 reduce feeds bf16 matmul"):
            if NODE_RED == "gp":
                hred = hid.tile([128, 2 * 2 * GT, NDATA // 2], BF16, tag="hred")
                nc.gpsimd.tensor_tensor(
                    hred, h2r[:, :, 0:NDATA // 2],
                    h2r[:, :, NDATA // 2:NDATA], ADD)
                nc.vector.reduce_sum(
                    out=hsum[:, :, o0:o0 + 2 * GT],
                    in_=hred.rearrange("c (k g) d -> c k g d", k=2), axis=AXX)
            else:
                nc.vector.reduce_sum(
                    out=hsum[:, :, o0:o0 + 2 * GT],
                    in_=h2r.rearrange("c (k g) d -> c k g d", k=2), axis=AXX)


def _node_sp_fe(nc, pools, w, xT_d, feT_raw, q):
    """One node super-pair with L3 applied BEFORE the datapoint sum:
    feT = sum_d(W3^T h2) per graph. L3 runs on the PE per slice and the
    d-sum reduces straight from PSUM on DVE, so no gpsimd/hsum tail chain.
    Writes feT_raw[:, q*64:(q+1)*64] (f32, bias not yet applied)."""
    consts, xin, hid, oot, psA, psB = pools
    rts = []
    for i in (0, 1):
        p = 2 * q + i
        xt = xin.tile([128, 2, TN], BF16, tag="xt")
        nc.sync.dma_start(
            xt, xT_d[:, p * PN:(p + 1) * PN].rearrange("c (t e) -> c t e", t=2))
        rts.append(xt)
    h2s = _super_pair(nc, pools, rts, w["nw1"], w["nw2"], w["nb1"], w["nb2"],
                      NODE_H1E, H2E)
    for i in (0, 1):
        p = 2 * q + i
        for t in (0, 1):
            s = 2 * p + t              # node slice index: graphs [16s, 16s+16)
            ps3 = psB.tile([ODE, GT, NDATA], F32, tag="l2", name=f"ps3_{i}{t}")
            for k in (0, 1):
                nc.tensor.matmul(ps3.rearrange("c g d -> c (g d)"),
                                 w["nw3"][:, k], h2s[i][:, k, t],
                                 start=(k == 0), stop=(k == 1))
            nc.vector.reduce_sum(
                out=feT_raw[:, s * GT:(s + 1) * GT], in_=ps3, axis=AXX)


def _edge_sp(nc, pools, w, attrT_d, outT_d, fe_src, q):
    """One edge super-pair: edges [q*2*PN, (q+1)*2*PN), 16 graphs.
    fe_src: ("sbuf", feT_sb) or ("dram", feTg_d)."""
    consts, xin, hid, oot, psA, psB = pools
    if True:
        rts = []
        for i in (0, 1):
            p = 2 * q + i
            e0 = p * PN
            rt = xin.tile([128, 2, TN], BF16, tag="rt")
            nc.sync.dma_start(
                rt[64:128], attrT_d[:, e0:e0 + PN].rearrange("c (t e) -> c t e", t=2))
            if fe_src[0] == "sbuf":
                _bcast_fe(nc, p, rt, fe_src[1])
            else:
                nc.sync.dma_start(
                    rt[0:64],
                    fe_src[1][:, e0:e0 + PN].rearrange("c (t e) -> c t e", t=2))
            rts.append(rt)
        h2s = _super_pair(nc, pools, rts, w["ew1"], w["ew2"], w["eb1"], w["eb2"],
                          EDGE_H1E, H2E)
        for i in (0, 1):
            p = 2 * q + i
            e0 = p * PN
            # L3 of the two slices in a pair share one PSUM bank via column
            # tiling (partitions 0:64 / 64:128), single accumulation group,
            # k-outer so the two column-group matmuls can run concurrently.
            # Allocated from the l2 tag: its slot-reuse WAR coincides with the
            # natural h2 dependency.
            l3 = psB.tile([128, TN], F32, tag="l2")
            # k-outer so each column-group's LDWEIGHTS overlaps the other
            # group's matmul. start=True on BOTH k0 matmuls: the has_written
            # clear is partition-scoped, so each clears only its own 64
            # partitions (whole-bank clearing would break this ordering).
            for k in (0, 1):
                for t in (0, 1):
                    nc.tensor.matmul(l3[64 * t:64 * (t + 1), :], w["ew3"][:, k],
                                     h2s[i][:, k, t],
                                     start=(k == 0), stop=(k == 1),
                                     skip_group_check=True)
            ot = oot.tile([128, TN], BF16, tag="ot")
            _drain(nc, "A" if p % 2 else "V", ot, l3, w["eb3x"], relu=False)
            nc.sync.dma_start(outT_d[:, e0:e0 + TN], ot[0:64])
            nc.sync.dma_start(outT_d[:, e0 + TN:e0 + PN], ot[64:128])


def _build(mode):
    """mode: 'fused' (node+edge, fe on-chip), 'node', 'edge'."""
    nc = bacc.Bacc("TRN2", target_bir_lowering=False)
    td = _declare_weights(nc)
    if mode in ("fused", "node"):
        xT_d = nc.dram_tensor("xT", [128, RC], BF16, kind="ExternalInput")
    if mode in ("fused", "edge"):
        attrT_d = nc.dram_tensor("attrT", [64, EC], BF16, kind="ExternalInput")
        outT_d = nc.dram_tensor("outT", [64, EC], BF16, kind="ExternalOutput")
    if mode == "edge":
        feTg_d = nc.dram_tensor("feTg", [64, EC], BF16, kind="ExternalInput")
    if mode == "node":
        feT_out = nc.dram_tensor("feT", [ODE, GC], F32, kind="ExternalOutput")

    with tile.TileContext(nc) as tc:
        with (
            tc.tile_pool(name="consts", bufs=1) as consts,
            tc.tile_pool(name="xin", bufs=16) as xin,
            tc.tile_pool(name="hid", bufs=8) as hid,
            tc.tile_pool(name="oot", bufs=10) as oot,
            tc.tile_pool(name="psA", bufs=4, space="PSUM") as psA,
            tc.tile_pool(name="psB", bufs=2, space="PSUM") as psB,
        ):
            pools = (consts, xin, hid, oot, psA, psB)
            w = _load_weights(nc, consts, td,
                              node=mode in ("fused", "node"),
                              edge=mode in ("fused", "edge"))
            NQN = RC // (2 * PN)   # node super-pairs (8); 64 graphs each
            NQE = EC // (2 * PN)   # edge super-pairs (32); 16 graphs each
            if mode == "node":
                hsum = consts.tile([128, 2, GC], BF16, tag="hsum")
            if mode == "fused":
                feT_raw = consts.tile([ODE, GC], F32, tag="feT_raw")
                for q in range(NQN):
                    _node_sp_fe(nc, pools, w, xT_d, feT_raw, q)
                feT_sb = consts.tile([ODE, GC], BF16, tag="feT")
                nc.scalar.activation(feT_sb, feT_raw, IDENT, bias=w["nb3"])
                for q in range(NQE):
                    _edge_sp(nc, pools, w, attrT_d, outT_d, ("sbuf", feT_sb), q)
            elif mode == "node":
                for q in range(NQN):
                    _node_sp(nc, pools, w, xT_d, hsum, q)
                ps_f = psB.tile([ODE, TN], F32, tag="l2")
                for k in (0, 1):
                    nc.tensor.matmul(ps_f, w["nw3"][:, k], hsum[:, k],
                                     start=(k == 0), stop=(k == 1))
                feT_sb = consts.tile([ODE, GC], F32, tag="feT")
                nc.scalar.activation(feT_sb, ps_f, IDENT, bias=w["nb3"])
                nc.sync.dma_start(feT_out[:], feT_sb)
            elif mode == "edge":
                for q in range(NQE):
                    _edge_sp(nc, pools, w, attrT_d, outT_d, ("dram", feTg_d), q)
    nc.finalize()
    return nc


def _get_program(mode):
    if mode not in _PROGRAMS:
        _PROGRAMS[mode] = _build(mode)
    return _PROGRAMS[mode]


def _shared_weight_arrays(kw):
    f = np.float32
    c = np.ascontiguousarray
    eb3 = np.asarray(kw["edge_b3"], dtype=f)
    return {
        "nw1": c(np.asarray(kw["node_w1"], dtype=f).astype(BF16NP)),
        "nw2": c(np.asarray(kw["node_w2"], dtype=f).reshape(2, 128, H).transpose(1, 0, 2).astype(BF16NP)),
        "nw3": c(np.asarray(kw["node_w3"], dtype=f).reshape(2, 128, ODE).transpose(1, 0, 2).astype(BF16NP)),
        "nb1": c(np.asarray(kw["node_b1"], dtype=f).reshape(2, 128).T),
        "nb2": c(np.asarray(kw["node_b2"], dtype=f).reshape(2, 128).T),
        "nb3": c(np.asarray(kw["node_b3"], dtype=f).reshape(ODE, 1)),
        "ew1": c(np.asarray(kw["edge_w1"], dtype=f).astype(BF16NP)),
        "ew2": c(np.asarray(kw["edge_w2"], dtype=f).reshape(2, 128, H).transpose(1, 0, 2).astype(BF16NP)),
        "ew3": c(np.asarray(kw["edge_w3"], dtype=f).reshape(2, 128, ODE).transpose(1, 0, 2).astype(BF16NP)),
        "eb1": c(np.asarray(kw["edge_b1"], dtype=f).reshape(2, 128).T),
        "eb2": c(np.asarray(kw["edge_b2"], dtype=f).reshape(2, 128).T),
        "eb3x": c(np.concatenate([eb3, eb3]).reshape(128, 1)),
    }


def _x_transposed_per_core(x, c):
    xs = np.asarray(x, dtype=np.float32).reshape(G, ODE, 2, NDATA)[c * GC:(c + 1) * GC]
    return np.ascontiguousarray(xs.transpose(1, 2, 0, 3).reshape(128, RC).astype(BF16NP))


def kernel(x, edge_attr, node_w1, node_b1, node_w2, node_b2, node_w3, node_b3,
           edge_w1, edge_b1, edge_w2, edge_b2, edge_w3, edge_b3,
           edge_index, batch):
    global last_results
    kw = dict(x=x, node_w1=node_w1, node_b1=node_b1, node_w2=node_w2,
              node_b2=node_b2, node_w3=node_w3, node_b3=node_b3,
              edge_w1=edge_w1, edge_b1=edge_b1, edge_w2=edge_w2,
              edge_b2=edge_b2, edge_w3=edge_w3, edge_b3=edge_b3)
    trace = os.environ.get("KERNEL_TRACE", "") == "1"
    if trace:
        _install_trace_shim()

    edge_attr = np.asarray(edge_attr, dtype=np.float32)
    ei = np.asarray(edge_index)
    bt = np.asarray(batch)
    g_src = bt[ei[0]]
    g_dst = bt[ei[1]]
    same = g_src == g_dst
    structured = bool((g_src == np.repeat(np.arange(G), EPG)).all())

    shared = _shared_weight_arrays(kw)
    run_kwargs = dict(core_ids=list(range(NCORES)), trace=trace,
                      trace_cores=[0] if trace else None)

    def _attr_perm(c):
        sl = edge_attr[c * EC:(c + 1) * EC][_ECOL]
        return np.ascontiguousarray(sl.T.astype(BF16NP))

    def _out_unperm(outT):
        o = np.empty((EC, EA), dtype=np.float32)
        o[_ECOL] = np.asarray(outT, dtype=np.float32).T
        return o

    if structured:
        nc = _get_program("fused")
        in_maps = []
        for c in range(NCORES):
            m = dict(shared)
            m["xT"] = _x_transposed_per_core(x, c)
            m["attrT"] = _attr_perm(c)
            in_maps.append(m)
        res = run_bass_kernel_spmd(nc, in_maps, **run_kwargs)
        last_results = res
        out = np.empty((E, EA), dtype=np.float32)
        for c in range(NCORES):
            out[c * EC:(c + 1) * EC] = _out_unperm(res.results[c]["outT"])
    else:
        # general path: node stage -> host gather of feature_enc -> edge stage
        nc_node = _get_program("node")
        in_maps = []
        for c in range(NCORES):
            m = dict(shared)
            m["xT"] = _x_transposed_per_core(x, c)
            in_maps.append(m)
        res_n = run_bass_kernel_spmd(nc_node, in_maps, **run_kwargs)
        feT_full = np.concatenate([np.asarray(res_n.results[c]["feT"], dtype=np.float32)
                                   for c in range(NCORES)], axis=1)  # [64, G]
        nc_edge = _get_program("edge")
        in_maps = []
        for c in range(NCORES):
            m = dict(shared)
            m["attrT"] = _attr_perm(c)
            gs = g_src[c * EC:(c + 1) * EC][_ECOL]
            m["feTg"] = np.ascontiguousarray(feT_full[:, gs].astype(BF16NP))
            in_maps.append(m)
        res = run_bass_kernel_spmd(nc_edge, in_maps, **run_kwargs)
        last_results = res
        out = np.empty((E, EA), dtype=np.float32)
        for c in range(NCORES):
            out[c * EC:(c + 1) * EC] = _out_unperm(res.results[c]["outT"])

    if not same.all():
        out = np.where(same[:, None], out, edge_attr)
    return out


# revision 35
# speedup vs baseline: 1.0175x; 1.0175x over previous
"""Trainium2 Bass kernel for nn_NodeNet (GNN message passing).

Strategy: data-parallel over graphs across 8 NeuronCores. Host transposes
inputs into [feature, row] layouts so every DMA is contiguous; all matmuls
run in bf16 with transposed activations:
  node stage: dp[128, rows] -> MLP -> sum over datapoints -> feature_enc[64, G]
  edge stage: rhs = [fe (broadcast per graph); edge_attr^T] -> MLP -> out^T

Structure: 512-column matmul slices, grouped in pairs (1024 cols) and
super-pairs (2048 cols). Within a super-pair matmuls are emitted
weight-major so each stationary operand loads once (LDWEIGHTS switches
halved). PSUM: L1 uses per-slice single-bank tiles (low-latency drains,
3-deep rotation); L2 uses pair-merged [128, 2, 512] two-bank tiles whose
halves hold the same feature chunk of both slices, so one ScalarE/VectorE
op drains 1024 columns with a single per-partition bias; L3 outputs of a
pair share one bank via column tiling (partitions 0-63 / 64-127) with a
single k-outer accumulation group. Drains alternate ScalarE/VectorE;
the per-graph feature_enc broadcast rotates gpsimd/DVE/ACT. Edge columns
are host-permuted into (rep, graph) interleaved order so the broadcast
has a contiguous innermost axis; host un-permutes the output. Outputs
store as bf16 and are upcast on host.

The structured fast path (edges grouped 128-per-graph, all within-graph,
as produced by the reference's setup_inputs) runs one fused launch per
core with feature_enc kept on-chip. A general fallback handles arbitrary
edge_index / batch with two launches and a host-side gather + mask.
"""

import os
import sys

import ml_dtypes
import numpy as np

BF16NP = ml_dtypes.bfloat16

if "/opt/trn_rl_repo" not in sys.path and os.path.isdir("/opt/trn_rl_repo"):
    sys.path.insert(0, "/opt/trn_rl_repo")

import concourse.bacc as bacc
import concourse.tile as tile
from concourse import mybir
from concourse.bass_utils import run_bass_kernel_spmd

G, ODE, NDATA, H, EA, EPG = 4096, 64, 32, 256, 64, 128
E = G * EPG
NCORES = 8
GC = G // NCORES           # graphs per core
RC = GC * NDATA            # node-MLP rows per core
EC = GC * EPG              # edges per core
TN = 512                   # matmul slice free size (one PSUM bank of fp32)
PN = 2 * TN                # pair free size
GT = TN // NDATA           # graphs covered per node slice (16)
GEP = PN // EPG            # graphs covered per edge pair (8)

F32 = mybir.dt.float32
BF16 = mybir.dt.bfloat16
RELU = mybir.ActivationFunctionType.Relu
IDENT = mybir.ActivationFunctionType.Identity
ADD = mybir.AluOpType.add
MAX = mybir.AluOpType.max
BYPASS = mybir.AluOpType.bypass
AXX = mybir.AxisListType.X

FEBCAST = os.environ.get("FEBCAST", "va")  # va | rot | gpsimd | ve | act
NODE_RED = os.environ.get("NODE_RED", "gp")  # gp (gpsimd TT halving) | ve

_PROGRAMS = {}
last_results = None


def _edge_col_perm():
    """Column -> edge mapping for the interleaved edge ordering.

    Within each pair p of matmul slices (PN=1024 columns, GEP=8 graphs),
    column j holds edge (rep r = j // GEP) of graph (g = j % GEP), i.e.
    local edge (8p + g) * EPG + r.
    """
    col = np.arange(EC)
    p, j = col // PN, col % PN
    r, g = j // GEP, j % GEP
    return (GEP * p + g) * EPG + r


_ECOL = _edge_col_perm()


def _install_trace_shim():
    """Optional: make trace=True work by injecting antenv.axon_hooks."""
    import types

    if "antenv.axon_hooks" in sys.modules:
        return
    try:
        mod = types.ModuleType("antenv.axon_hooks")
        mod._hook = None
        mod.set_axon_ntff_profile_hook = lambda h: setattr(mod, "_hook", h)
        mod.get_axon_ntff_profile_hook = lambda: mod._hook
        sys.modules["antenv.axon_hooks"] = mod
        import antenv

        antenv.axon_hooks = mod
        from trn_agent_boot.trn_boot import _ntff_profile_via_ctypes

        hook = _ntff_profile_via_ctypes("/opt/axon/libaxon_pjrt.so")
        if hook is not None:
            mod.set_axon_ntff_profile_hook(hook)
    except Exception:
        pass


def _declare_weights(nc):
    t = {}
    t["nw1"] = nc.dram_tensor("nw1", [128, H], BF16, kind="ExternalInput")
    t["nw2"] = nc.dram_tensor("nw2", [128, 2, H], BF16, kind="ExternalInput")
    t["nw3"] = nc.dram_tensor("nw3", [128, 2, ODE], BF16, kind="ExternalInput")
    t["nb1"] = nc.dram_tensor("nb1", [128, 2], F32, kind="ExternalInput")
    t["nb2"] = nc.dram_tensor("nb2", [128, 2], F32, kind="ExternalInput")
    t["nb3"] = nc.dram_tensor("nb3", [ODE, 1], F32, kind="ExternalInput")
    t["ew1"] = nc.dram_tensor("ew1", [128, H], BF16, kind="ExternalInput")
    t["ew2"] = nc.dram_tensor("ew2", [128, 2, H], BF16, kind="ExternalInput")
    t["ew3"] = nc.dram_tensor("ew3", [128, 2, ODE], BF16, kind="ExternalInput")
    t["eb1"] = nc.dram_tensor("eb1", [128, 2], F32, kind="ExternalInput")
    t["eb2"] = nc.dram_tensor("eb2", [128, 2], F32, kind="ExternalInput")
    t["eb3x"] = nc.dram_tensor("eb3x", [128, 1], F32, kind="ExternalInput")
    return t


def _load_weights(nc, consts, td, node: bool, edge: bool):
    sb = {}
    names = []
    if node:
        names += ["nw1", "nw2", "nw3", "nb1", "nb2", "nb3"]
    if edge:
        names += ["ew1", "ew2", "ew3", "eb1", "eb2", "eb3x"]
    for n in names:
        d = td[n]
        sb[n] = consts.tile(list(d.shape), d.dtype, tag=n, name=n)
        nc.sync.dma_start(sb[n], d[:])
    return sb


def _drain(nc, engine, out, psum, bias, relu=True):
    """PSUM -> SBUF with bias add (+ optional relu) on the chosen engine."""
    if engine == "A":
        nc.scalar.activation(out, psum, RELU if relu else IDENT, bias=bias)
    else:
        nc.vector.tensor_scalar(
            out=out, in0=psum, scalar1=bias, scalar2=0.0,
            op0=ADD, op1=MAX if relu else BYPASS,
        )


def _bcast_fe(nc, p, rt, feT_sb):
    """Fill rt[0:64] with feature_enc of this pair's GEP graphs, repeated
    per edge (interleaved ordering: innermost axis is the GEP graphs)."""
    bc_out = rt[0:64].rearrange("c t (r g) -> c (t r) g", g=GEP)
    bc_in = feT_sb[:, None, p * GEP:(p + 1) * GEP].to_broadcast(
        [ODE, PN // GEP, GEP])
    mode = FEBCAST
    if mode == "rot":
        mode = ("gpsimd", "ve", "act")[p % 3]
    elif mode == "va":
        mode = ("ve", "act")[p % 2]
    if mode == "gpsimd":
        nc.gpsimd.tensor_copy(out=bc_out, in_=bc_in)
    elif mode == "act":
        nc.scalar.copy(bc_out, bc_in)
    else:
        nc.vector.tensor_copy(out=bc_out, in_=bc_in)


def _super_pair(nc, pools, rts, w1, w2, b1, b2, h1_engines, h2_engines):
    """Layers 1+2 for a super-pair (two pairs), emitted weight-major.

    rts: two rhs tiles [128, 2, TN]. Returns two h2 tiles
    [128, 2(chunk), 2(slice), TN] in bf16."""
    consts, xin, hid, oot, psA, psB = pools
    l1 = {}
    for c in (0, 1):
        for i in (0, 1):
            for t in (0, 1):
                l1[(i, c, t)] = psA.tile([128, TN], F32, tag="l1",
                                         name=f"l1_{i}{c}{t}")
    # weight-major L1: one LDWEIGHTS per chunk for all four slices
    for c in (0, 1):
        for i in (0, 1):
            for t in (0, 1):
                nc.tensor.matmul(l1[(i, c, t)], w1[:, 128 * c:128 * (c + 1)],
                                 rts[i][:, t], start=True, stop=True)
    h1s = [hid.tile([128, 2, 2, TN], BF16, tag="h1", name=f"h1_{i}")
           for i in (0, 1)]
    for c in (0, 1):
        for i in (0, 1):
            for t in (0, 1):
                _drain(nc, h1_engines[(i, c, t)], h1s[i][:, c, t], l1[(i, c, t)],
                       b1[:, c:c + 1])
    l2 = {}
    for c in (0, 1):
        for i in (0, 1):
            l2[(i, c)] = psB.tile([128, 2, TN], F32, tag="l2",
                                  name=f"l2_{i}{c}")
    # weight-major L2: one LDWEIGHTS per (k, chunk) for all four slices
    for c in (0, 1):
        for k in (0, 1):
            for i in (0, 1):
                for t in (0, 1):
                    nc.tensor.matmul(l2[(i, c)][:, t], w2[:, k, 128 * c:128 * (c + 1)],
                                     h1s[i][:, k, t], start=(k == 0), stop=(k == 1))
    h2s = [hid.tile([128, 2, 2, TN], BF16, tag="h2", name=f"h2_{i}")
           for i in (0, 1)]
    for c in (0, 1):
        for i in (0, 1):
            _drain(nc, h2_engines[(i, c)], h2s[i][:, c], l2[(i, c)], b2[:, c:c + 1])
    return h2s


NODE_H1E = {(0, 0, 0): "A", (0, 0, 1): "A", (0, 1, 0): "V", (0, 1, 1): "A",
            (1, 0, 0): "A", (1, 0, 1): "A", (1, 1, 0): "V", (1, 1, 1): "A"}
EDGE_H1E = {(0, 0, 0): "A", (0, 0, 1): "A", (0, 1, 0): "V", (0, 1, 1): "V",
            (1, 0, 0): "A", (1, 0, 1): "A", (1, 1, 0): "V", (1, 1, 1): "V"}
H2E = {(0, 0): "A", (0, 1): "V", (1, 0): "A", (1, 1): "V"}


def _node_sp(nc, pools, w, xT_d, hsum, q):
    """One node super-pair: rows [q*2*PN, (q+1)*2*PN), 64 graphs. Writes
    hsum[:, :, q*64:(q+1)*64]."""
    consts, xin, hid, oot, psA, psB = pools
    rts = []
    for i in (0, 1):
        p = 2 * q + i
        xt = xin.tile([128, 2, TN], BF16, tag="xt")
        nc.sync.dma_start(
            xt, xT_d[:, p * PN:(p + 1) * PN].rearrange("c (t e) -> c t e", t=2))
        rts.append(xt)
    h2s = _super_pair(nc, pools, rts, w["nw1"], w["nw2"], w["nb1"], w["nb2"],
                      NODE_H1E, H2E)
    for i in (0, 1):
        p = 2 * q + i
        h2r = h2s[i].rearrange("c k t (g d) -> c (k t g) d", d=NDATA)
        o0 = p * 2 * GT
        with nc.allow_low_precision(reason="bf16 reduce feeds bf16 matmul"):
            if NODE_RED == "gp":
                hred = hid.tile([128, 2 * 2 * GT, NDATA // 2], BF16, tag="hred")
                nc.gpsimd.tensor_tensor(
                    hred, h2r[:, :, 0:NDATA // 2],
                    h2r[:, :, NDATA // 2:NDATA], ADD)
                nc.vector.reduce_sum(
                    out=hsum[:, :, o0:o0 + 2 * GT],
                    in_=hred.rearrange("c (k g) d -> c k g d", k=2), axis=AXX)
            else:
                nc.vector.reduce_sum(
                    out=hsum[:, :, o0:o0 + 2 * GT],
                    in_=h2r.rearrange("c (k g) d -> c k g d", k=2), axis=AXX)


def _node_sp_fe(nc, pools, w, xT_d, feT_raw, q):
    """One node super-pair with L3 applied BEFORE the datapoint sum:
    feT = sum_d(W3^T h2) per graph. L3 runs on the PE per slice and the
    d-sum reduces straight from PSUM on DVE, so no gpsimd/hsum tail chain.
    Writes feT_raw[:, q*64:(q+1)*64] (f32, bias not yet applied)."""
    consts, xin, hid, oot, psA, psB = pools
    rts = []
    for i in (0, 1):
        p = 2 * q + i
        xt = xin.tile([128, 2, TN], BF16, tag="xt")
        nc.sync.dma_start(
            xt, xT_d[:, p * PN:(p + 1) * PN].rearrange("c (t e) -> c t e", t=2))
        rts.append(xt)
    h2s = _super_pair(nc, pools, rts, w["nw1"], w["nw2"], w["nb1"], w["nb2"],
                      NODE_H1E, H2E)
    for i in (0, 1):
        p = 2 * q + i
        for t in (0, 1):
            s = 2 * p + t              # node slice index: graphs [16s, 16s+16)
            ps3 = psB.tile([ODE, GT, NDATA], F32, tag="l2", name=f"ps3_{i}{t}")
            for k in (0, 1):
                nc.tensor.matmul(ps3.rearrange("c g d -> c (g d)"),
                                 w["nw3"][:, k], h2s[i][:, k, t],
                                 start=(k == 0), stop=(k == 1))
            nc.vector.reduce_sum(
                out=feT_raw[:, s * GT:(s + 1) * GT], in_=ps3, axis=AXX)


def _edge_sp(nc, pools, w, attrT_d, outT_d, fe_src, q):
    """One edge super-pair: edges [q*2*PN, (q+1)*2*PN), 16 graphs.
    fe_src: ("sbuf", feT_sb) or ("dram", feTg_d)."""
    consts, xin, hid, oot, psA, psB = pools
    if True:
        rts = []
        for i in (0, 1):
            p = 2 * q + i
            e0 = p * PN
            rt = xin.tile([128, 2, TN], BF16, tag="rt")
            nc.sync.dma_start(
                rt[64:128], attrT_d[:, e0:e0 + PN].rearrange("c (t e) -> c t e", t=2))
            if fe_src[0] == "sbuf":
                _bcast_fe(nc, p, rt, fe_src[1])
            else:
                nc.sync.dma_start(
                    rt[0:64],
                    fe_src[1][:, e0:e0 + PN].rearrange("c (t e) -> c t e", t=2))
            rts.append(rt)
        h2s = _super_pair(nc, pools, rts, w["ew1"], w["ew2"], w["eb1"], w["eb2"],
                          EDGE_H1E, H2E)
        for i in (0, 1):
            p = 2 * q + i
            e0 = p * PN
            # L3 of the two slices in a pair share one PSUM bank via column
            # tiling (partitions 0:64 / 64:128), single accumulation group,
            # k-outer so the two column-group matmuls can run concurrently.
            # Allocated from the l2 tag: its slot-reuse WAR coincides with the
            # natural h2 dependency.
            l3 = psB.tile([128, TN], F32, tag="l2")
            # k-outer so each column-group's LDWEIGHTS overlaps the other
            # group's matmul. start=True on BOTH k0 matmuls: the has_written
            # clear is partition-scoped, so each clears only its own 64
            # partitions (whole-bank clearing would break this ordering).
            for k in (0, 1):
                for t in (0, 1):
                    nc.tensor.matmul(l3[64 * t:64 * (t + 1), :], w["ew3"][:, k],
                                     h2s[i][:, k, t],
                                     start=(k == 0), stop=(k == 1),
                                     skip_group_check=True)
            ot = oot.tile([128, TN], BF16, tag="ot")
            _drain(nc, "A" if p % 2 else "V", ot, l3, w["eb3x"], relu=False)
            nc.sync.dma_start(outT_d[:, e0:e0 + TN], ot[0:64])
            nc.sync.dma_start(outT_d[:, e0 + TN:e0 + PN], ot[64:128])


def _build(mode):
    """mode: 'fused' (node+edge, fe on-chip), 'node', 'edge'."""
    nc = bacc.Bacc("TRN2", target_bir_lowering=False)
    td = _declare_weights(nc)
    if mode in ("fused", "node"):
        xT_d = nc.dram_tensor("xT", [128, RC], BF16, kind="ExternalInput")
    if mode in ("fused", "edge"):
        attrT_d = nc.dram_tensor("attrT", [64, EC], BF16, kind="ExternalInput")
        outT_d = nc.dram_tensor("outT", [64, EC], BF16, kind="ExternalOutput")
    if mode == "edge":
        feTg_d = nc.dram_tensor("feTg", [64, EC], BF16, kind="ExternalInput")
    if mode == "node":
        feT_out = nc.dram_tensor("feT", [ODE, GC], F32, kind="ExternalOutput")

    with tile.TileContext(nc) as tc:
        with (
            tc.tile_pool(name="consts", bufs=1) as consts,
            tc.tile_pool(name="xin", bufs=12) as xin,
            tc.tile_pool(name="hid", bufs=6) as hid,
            tc.tile_pool(name="oot", bufs=8) as oot,
            tc.tile_pool(name="psA", bufs=4, space="PSUM") as psA,
            tc.tile_pool(name="psB", bufs=2, space="PSUM") as psB,
        ):
            pools = (consts, xin, hid, oot, psA, psB)
            w = _load_weights(nc, consts, td,
                              node=mode in ("fused", "node"),
                              edge=mode in ("fused", "edge"))
            NQN = RC // (2 * PN)   # node super-pairs (8); 64 graphs each
            NQE = EC // (2 * PN)   # edge super-pairs (32); 16 graphs each
            if mode == "node":
                hsum = consts.tile([128, 2, GC], BF16, tag="hsum")
            if mode == "fused":
                feT_raw = consts.tile([ODE, GC], F32, tag="feT_raw")
                for q in range(NQN):
                    _node_sp_fe(nc, pools, w, xT_d, feT_raw, q)
                feT_sb = consts.tile([ODE, GC], BF16, tag="feT")
                nc.scalar.activation(feT_sb, feT_raw, IDENT, bias=w["nb3"])
                for q in range(NQE):
                    _edge_sp(nc, pools, w, attrT_d, outT_d, ("sbuf", feT_sb), q)
            elif mode == "node":
                for q in range(NQN):
                    _node_sp(nc, pools, w, xT_d, hsum, q)
                ps_f = psB.tile([ODE, TN], F32, tag="l2")
                for k in (0, 1):
                    nc.tensor.matmul(ps_f, w["nw3"][:, k], hsum[:, k],
                                     start=(k == 0), stop=(k == 1))
                feT_sb = consts.tile([ODE, GC], F32, tag="feT")
                nc.scalar.activation(feT_sb, ps_f, IDENT, bias=w["nb3"])
                nc.sync.dma_start(feT_out[:], feT_sb)
            elif mode == "edge":
                for q in range(NQE):
                    _edge_sp(nc, pools, w, attrT_d, outT_d, ("dram", feTg_d), q)
    nc.finalize()
    return nc


def _get_program(mode):
    if mode not in _PROGRAMS:
        _PROGRAMS[mode] = _build(mode)
    return _PROGRAMS[mode]


def _shared_weight_arrays(kw):
    f = np.float32
    c = np.ascontiguousarray
    eb3 = np.asarray(kw["edge_b3"], dtype=f)
    return {
        "nw1": c(np.asarray(kw["node_w1"], dtype=f).astype(BF16NP)),
        "nw2": c(np.asarray(kw["node_w2"], dtype=f).reshape(2, 128, H).transpose(1, 0, 2).astype(BF16NP)),
        "nw3": c(np.asarray(kw["node_w3"], dtype=f).reshape(2, 128, ODE).transpose(1, 0, 2).astype(BF16NP)),
        "nb1": c(np.asarray(kw["node_b1"], dtype=f).reshape(2, 128).T),
        "nb2": c(np.asarray(kw["node_b2"], dtype=f).reshape(2, 128).T),
        "nb3": c(np.asarray(kw["node_b3"], dtype=f).reshape(ODE, 1)),
        "ew1": c(np.asarray(kw["edge_w1"], dtype=f).astype(BF16NP)),
        "ew2": c(np.asarray(kw["edge_w2"], dtype=f).reshape(2, 128, H).transpose(1, 0, 2).astype(BF16NP)),
        "ew3": c(np.asarray(kw["edge_w3"], dtype=f).reshape(2, 128, ODE).transpose(1, 0, 2).astype(BF16NP)),
        "eb1": c(np.asarray(kw["edge_b1"], dtype=f).reshape(2, 128).T),
        "eb2": c(np.asarray(kw["edge_b2"], dtype=f).reshape(2, 128).T),
        "eb3x": c(np.concatenate([eb3, eb3]).reshape(128, 1)),
    }


def _x_transposed_per_core(x, c):
    xs = np.asarray(x, dtype=np.float32).reshape(G, ODE, 2, NDATA)[c * GC:(c + 1) * GC]
    return np.ascontiguousarray(xs.transpose(1, 2, 0, 3).reshape(128, RC).astype(BF16NP))


def kernel(x, edge_attr, node_w1, node_b1, node_w2, node_b2, node_w3, node_b3,
           edge_w1, edge_b1, edge_w2, edge_b2, edge_w3, edge_b3,
           edge_index, batch):
    global last_results
    kw = dict(x=x, node_w1=node_w1, node_b1=node_b1, node_w2=node_w2,
              node_b2=node_b2, node_w3=node_w3, node_b3=node_b3,
              edge_w1=edge_w1, edge_b1=edge_b1, edge_w2=edge_w2,
              edge_b2=edge_b2, edge_w3=edge_w3, edge_b3=edge_b3)
    trace = os.environ.get("KERNEL_TRACE", "") == "1"
    if trace:
        _install_trace_shim()

    edge_attr = np.asarray(edge_attr, dtype=np.float32)
    ei = np.asarray(edge_index)
    bt = np.asarray(batch)
    g_src = bt[ei[0]]
    g_dst = bt[ei[1]]
    same = g_src == g_dst
    structured = bool((g_src == np.repeat(np.arange(G), EPG)).all())

    shared = _shared_weight_arrays(kw)
    run_kwargs = dict(core_ids=list(range(NCORES)), trace=trace,
                      trace_cores=[0] if trace else None)

    def _attr_perm(c):
        sl = edge_attr[c * EC:(c + 1) * EC][_ECOL]
        return np.ascontiguousarray(sl.T.astype(BF16NP))

    def _out_unperm(outT):
        o = np.empty((EC, EA), dtype=np.float32)
        o[_ECOL] = np.asarray(outT, dtype=np.float32).T
        return o

    if structured:
        nc = _get_program("fused")
        in_maps = []
        for c in range(NCORES):
            m = dict(shared)
            m["xT"] = _x_transposed_per_core(x, c)
            m["attrT"] = _attr_perm(c)
            in_maps.append(m)
        res = run_bass_kernel_spmd(nc, in_maps, **run_kwargs)
        last_results = res
        out = np.empty((E, EA), dtype=np.float32)
        for c in range(NCORES):
            out[c * EC:(c + 1) * EC] = _out_unperm(res.results[c]["outT"])
    else:
        # general path: node stage -> host gather of feature_enc -> edge stage
        nc_node = _get_program("node")
        in_maps = []
        for c in range(NCORES):
            m = dict(shared)
            m["xT"] = _x_transposed_per_core(x, c)
            in_maps.append(m)
        res_n = run_bass_kernel_spmd(nc_node, in_maps, **run_kwargs)
        feT_full = np.concatenate([np.asarray(res_n.results[c]["feT"], dtype=np.float32)
                                   for c in range(NCORES)], axis=1)  # [64, G]
        nc_edge = _get_program("edge")
        in_maps = []
        for c in range(NCORES):
            m = dict(shared)
            m["attrT"] = _attr_perm(c)
            gs = g_src[c * EC:(c + 1) * EC][_ECOL]
            m["feTg"] = np.ascontiguousarray(feT_full[:, gs].astype(BF16NP))
            in_maps.append(m)
        res = run_bass_kernel_spmd(nc_edge, in_maps, **run_kwargs)
        last_results = res
        out = np.empty((E, EA), dtype=np.float32)
        for c in range(NCORES):
            out[c * EC:(c + 1) * EC] = _out_unperm(res.results[c]["outT"])

    if not same.all():
        out = np.where(same[:, None], out, edge_attr)
    return out


# revision 40
# speedup vs baseline: 1.0490x; 1.0309x over previous
"""Trainium2 Bass kernel for nn_NodeNet (GNN message passing).

Strategy: data-parallel over graphs across 8 NeuronCores. Host transposes
inputs into [feature, row] layouts so every DMA is contiguous; all matmuls
run in bf16 with transposed activations:
  node stage: dp[128, rows] -> MLP -> sum over datapoints -> feature_enc[64, G]
  edge stage: rhs = [fe (broadcast per graph); edge_attr^T] -> MLP -> out^T

Structure: 512-column matmul slices, grouped in pairs (1024 cols) and
super-pairs (2048 cols). Within a super-pair matmuls are emitted
weight-major so each stationary operand loads once (LDWEIGHTS switches
halved). PSUM: L1 uses per-slice single-bank tiles (low-latency drains,
3-deep rotation); L2 uses pair-merged [128, 2, 512] two-bank tiles whose
halves hold the same feature chunk of both slices, so one ScalarE/VectorE
op drains 1024 columns with a single per-partition bias; L3 outputs of a
pair share one bank via column tiling (partitions 0-63 / 64-127) with a
single k-outer accumulation group. Drains alternate ScalarE/VectorE;
the per-graph feature_enc broadcast rotates gpsimd/DVE/ACT. Edge columns
are host-permuted into (rep, graph) interleaved order so the broadcast
has a contiguous innermost axis; host un-permutes the output. Outputs
store as bf16 and are upcast on host.

The structured fast path (edges grouped 128-per-graph, all within-graph,
as produced by the reference's setup_inputs) runs one fused launch per
core with feature_enc kept on-chip. A general fallback handles arbitrary
edge_index / batch with two launches and a host-side gather + mask.
"""

import os
import sys

import ml_dtypes
import numpy as np

BF16NP = ml_dtypes.bfloat16

if "/opt/trn_rl_repo" not in sys.path and os.path.isdir("/opt/trn_rl_repo"):
    sys.path.insert(0, "/opt/trn_rl_repo")

import concourse.bacc as bacc
import concourse.tile as tile
from concourse import mybir
from concourse.bass_utils import run_bass_kernel_spmd

G, ODE, NDATA, H, EA, EPG = 4096, 64, 32, 256, 64, 128
E = G * EPG
NCORES = 8
GC = G // NCORES           # graphs per core
RC = GC * NDATA            # node-MLP rows per core
EC = GC * EPG              # edges per core
TN = 512                   # matmul slice free size (one PSUM bank of fp32)
PN = 2 * TN                # pair free size
GT = TN // NDATA           # graphs covered per node slice (16)
GEP = PN // EPG            # graphs covered per edge pair (8)

F32 = mybir.dt.float32
BF16 = mybir.dt.bfloat16
RELU = mybir.ActivationFunctionType.Relu
IDENT = mybir.ActivationFunctionType.Identity
ADD = mybir.AluOpType.add
MAX = mybir.AluOpType.max
BYPASS = mybir.AluOpType.bypass
AXX = mybir.AxisListType.X

FEBCAST = os.environ.get("FEBCAST", "va")  # va | rot | gpsimd | ve | act
NODE_RED = os.environ.get("NODE_RED", "gp")  # gp (gpsimd TT halving) | ve

_PROGRAMS = {}
last_results = None


def _edge_col_perm():
    """Column -> edge mapping for the interleaved edge ordering.

    Within each pair p of matmul slices (PN=1024 columns, GEP=8 graphs),
    column j holds edge (rep r = j // GEP) of graph (g = j % GEP), i.e.
    local edge (8p + g) * EPG + r.
    """
    col = np.arange(EC)
    p, j = col // PN, col % PN
    r, g = j // GEP, j % GEP
    return (GEP * p + g) * EPG + r


_ECOL = _edge_col_perm()


def _install_trace_shim():
    """Optional: make trace=True work by injecting antenv.axon_hooks."""
    import types

    if "antenv.axon_hooks" in sys.modules:
        return
    try:
        mod = types.ModuleType("antenv.axon_hooks")
        mod._hook = None
        mod.set_axon_ntff_profile_hook = lambda h: setattr(mod, "_hook", h)
        mod.get_axon_ntff_profile_hook = lambda: mod._hook
        sys.modules["antenv.axon_hooks"] = mod
        import antenv

        antenv.axon_hooks = mod
        from trn_agent_boot.trn_boot import _ntff_profile_via_ctypes

        hook = _ntff_profile_via_ctypes("/opt/axon/libaxon_pjrt.so")
        if hook is not None:
            mod.set_axon_ntff_profile_hook(hook)
    except Exception:
        pass


def _declare_weights(nc):
    t = {}
    t["nw1"] = nc.dram_tensor("nw1", [128, H], BF16, kind="ExternalInput")
    t["nw2"] = nc.dram_tensor("nw2", [128, 2, H], BF16, kind="ExternalInput")
    t["nw3"] = nc.dram_tensor("nw3", [128, 2, ODE], BF16, kind="ExternalInput")
    t["nb1"] = nc.dram_tensor("nb1", [128, 2], F32, kind="ExternalInput")
    t["nb2"] = nc.dram_tensor("nb2", [128, 2], F32, kind="ExternalInput")
    t["nb3"] = nc.dram_tensor("nb3", [ODE, 1], F32, kind="ExternalInput")
    t["ew1"] = nc.dram_tensor("ew1", [128, H], BF16, kind="ExternalInput")
    t["ew2"] = nc.dram_tensor("ew2", [128, 2, H], BF16, kind="ExternalInput")
    t["ew3"] = nc.dram_tensor("ew3", [128, 2, ODE], BF16, kind="ExternalInput")
    t["eb1"] = nc.dram_tensor("eb1", [128, 2], F32, kind="ExternalInput")
    t["eb2"] = nc.dram_tensor("eb2", [128, 2], F32, kind="ExternalInput")
    t["eb3x"] = nc.dram_tensor("eb3x", [128, 1], F32, kind="ExternalInput")
    return t


def _load_weights(nc, consts, td, node: bool, edge: bool):
    sb = {}
    names = []
    if node:
        names += ["nw1", "nw2", "nw3", "nb1", "nb2", "nb3"]
    if edge:
        names += ["ew1", "ew2", "ew3", "eb1", "eb2", "eb3x"]
    for n in names:
        d = td[n]
        sb[n] = consts.tile(list(d.shape), d.dtype, tag=n, name=n)
        nc.sync.dma_start(sb[n], d[:])
    return sb


def _drain(nc, engine, out, psum, bias, relu=True):
    """PSUM -> SBUF with bias add (+ optional relu) on the chosen engine."""
    if engine == "A":
        nc.scalar.activation(out, psum, RELU if relu else IDENT, bias=bias)
    else:
        nc.vector.tensor_scalar(
            out=out, in0=psum, scalar1=bias, scalar2=0.0,
            op0=ADD, op1=MAX if relu else BYPASS,
        )


def _bcast_fe(nc, p, rt, feT_sb):
    """Fill rt[0:64] with feature_enc of this pair's GEP graphs, repeated
    per edge (interleaved ordering: innermost axis is the GEP graphs)."""
    bc_out = rt[0:64].rearrange("c t (r g) -> c (t r) g", g=GEP)
    bc_in = feT_sb[:, None, p * GEP:(p + 1) * GEP].to_broadcast(
        [ODE, PN // GEP, GEP])
    mode = FEBCAST
    if mode == "rot":
        mode = ("gpsimd", "ve", "act")[p % 3]
    elif mode == "va":
        mode = ("ve", "act")[p % 2]
    if mode == "gpsimd":
        nc.gpsimd.tensor_copy(out=bc_out, in_=bc_in)
    elif mode == "act":
        nc.scalar.copy(bc_out, bc_in)
    else:
        nc.vector.tensor_copy(out=bc_out, in_=bc_in)


def _super_pair(nc, pools, rts, w1, w2, b1, b2, h1_engines, h2_engines):
    """Layers 1+2 for a super-pair (two pairs), emitted weight-major.

    rts: two rhs tiles [128, 2, TN]. Returns two h2 tiles
    [128, 2(chunk), 2(slice), TN] in bf16."""
    consts, xin, hid, oot, psA, psB = pools
    l1 = {}
    for c in (0, 1):
        for i in (0, 1):
            for t in (0, 1):
                l1[(i, c, t)] = psA.tile([128, TN], F32, tag="l1",
                                         name=f"l1_{i}{c}{t}")
    # weight-major L1: one LDWEIGHTS per chunk for all four slices
    for c in (0, 1):
        for i in (0, 1):
            for t in (0, 1):
                nc.tensor.matmul(l1[(i, c, t)], w1[:, 128 * c:128 * (c + 1)],
                                 rts[i][:, t], start=True, stop=True)
    h1s = [hid.tile([128, 2, 2, TN], BF16, tag="h1", name=f"h1_{i}")
           for i in (0, 1)]
    for c in (0, 1):
        for i in (0, 1):
            for t in (0, 1):
                _drain(nc, h1_engines[(i, c, t)], h1s[i][:, c, t], l1[(i, c, t)],
                       b1[:, c:c + 1])
    l2 = {}
    for c in (0, 1):
        for i in (0, 1):
            l2[(i, c)] = psB.tile([128, 2, TN], F32, tag="l2",
                                  name=f"l2_{i}{c}")
    # weight-major L2: one LDWEIGHTS per (k, chunk) for all four slices
    for c in (0, 1):
        for k in (0, 1):
            for i in (0, 1):
                for t in (0, 1):
                    nc.tensor.matmul(l2[(i, c)][:, t], w2[:, k, 128 * c:128 * (c + 1)],
                                     h1s[i][:, k, t], start=(k == 0), stop=(k == 1))
    h2s = [hid.tile([128, 2, 2, TN], BF16, tag="h2", name=f"h2_{i}")
           for i in (0, 1)]
    for c in (0, 1):
        for i in (0, 1):
            _drain(nc, h2_engines[(i, c)], h2s[i][:, c], l2[(i, c)], b2[:, c:c + 1])
    return h2s


NODE_H1E = {(0, 0, 0): "A", (0, 0, 1): "A", (0, 1, 0): "V", (0, 1, 1): "A",
            (1, 0, 0): "A", (1, 0, 1): "A", (1, 1, 0): "V", (1, 1, 1): "A"}
EDGE_H1E = {(0, 0, 0): "A", (0, 0, 1): "A", (0, 1, 0): "V", (0, 1, 1): "V",
            (1, 0, 0): "A", (1, 0, 1): "A", (1, 1, 0): "V", (1, 1, 1): "V"}
H2E = {(0, 0): "A", (0, 1): "V", (1, 0): "A", (1, 1): "V"}


def _node_sp(nc, pools, w, xT_d, hsum, q, rts=None):
    """One node super-pair: rows [q*2*PN, (q+1)*2*PN), 64 graphs. Writes
    hsum[:, :, q*64:(q+1)*64]."""
    consts, xin, hid, oot, psA, psB = pools
    if rts is None:
        rts = [_xt_load(nc, pools, xT_d, 2 * q + i) for i in (0, 1)]
    h2s = _super_pair(nc, pools, rts, w["nw1"], w["nw2"], w["nb1"], w["nb2"],
                      NODE_H1E, H2E)
    for i in (0, 1):
        p = 2 * q + i
        h2r = h2s[i].rearrange("c k t (g d) -> c (k t g) d", d=NDATA)
        o0 = p * 2 * GT
        with nc.allow_low_precision(reason="bf16 reduce feeds bf16 matmul"):
            if NODE_RED == "gp":
                hred = hid.tile([128, 2 * 2 * GT, NDATA // 2], BF16, tag="hred")
                nc.gpsimd.tensor_tensor(
                    hred, h2r[:, :, 0:NDATA // 2],
                    h2r[:, :, NDATA // 2:NDATA], ADD)
                nc.vector.reduce_sum(
                    out=hsum[:, :, o0:o0 + 2 * GT],
                    in_=hred.rearrange("c (k g) d -> c k g d", k=2), axis=AXX)
            else:
                nc.vector.reduce_sum(
                    out=hsum[:, :, o0:o0 + 2 * GT],
                    in_=h2r.rearrange("c (k g) d -> c k g d", k=2), axis=AXX)


def _xt_load(nc, pools, xT_d, p):
    consts, xin, hid, oot, psA, psB = pools
    xt = xin.tile([128, 2, TN], BF16, tag="xt", name=f"xt{p}")
    nc.sync.dma_start(
        xt, xT_d[:, p * PN:(p + 1) * PN].rearrange("c (t e) -> c t e", t=2))
    return xt


def _node_sp_fe(nc, pools, w, xT_d, feT_raw, q, rts=None):
    """One node super-pair with L3 applied BEFORE the datapoint sum:
    feT = sum_d(W3^T h2) per graph. L3 runs on the PE per slice and the
    d-sum reduces straight from PSUM on DVE, so no gpsimd/hsum tail chain.
    Writes feT_raw[:, q*64:(q+1)*64] (f32, bias not yet applied)."""
    consts, xin, hid, oot, psA, psB = pools
    if rts is None:
        rts = [_xt_load(nc, pools, xT_d, 2 * q + i) for i in (0, 1)]
    h2s = _super_pair(nc, pools, rts, w["nw1"], w["nw2"], w["nb1"], w["nb2"],
                      NODE_H1E, H2E)
    for i in (0, 1):
        p = 2 * q + i
        for t in (0, 1):
            s = 2 * p + t              # node slice index: graphs [16s, 16s+16)
            ps3 = psB.tile([ODE, GT, NDATA], F32, tag="l2", name=f"ps3_{i}{t}")
            for k in (0, 1):
                nc.tensor.matmul(ps3.rearrange("c g d -> c (g d)"),
                                 w["nw3"][:, k], h2s[i][:, k, t],
                                 start=(k == 0), stop=(k == 1))
            nc.vector.reduce_sum(
                out=feT_raw[:, s * GT:(s + 1) * GT], in_=ps3, axis=AXX)


def _edge_sp(nc, pools, w, attrT_d, outT_d, fe_src, q):
    """One edge super-pair: edges [q*2*PN, (q+1)*2*PN), 16 graphs.
    fe_src: ("sbuf", feT_sb) or ("dram", feTg_d)."""
    consts, xin, hid, oot, psA, psB = pools
    if True:
        rts = []
        for i in (0, 1):
            p = 2 * q + i
            e0 = p * PN
            rt = xin.tile([128, 2, TN], BF16, tag="rt")
            nc.sync.dma_start(
                rt[64:128], attrT_d[:, e0:e0 + PN].rearrange("c (t e) -> c t e", t=2))
            if fe_src[0] == "sbuf":
                _bcast_fe(nc, p, rt, fe_src[1])
            else:
                nc.sync.dma_start(
                    rt[0:64],
                    fe_src[1][:, e0:e0 + PN].rearrange("c (t e) -> c t e", t=2))
            rts.append(rt)
        h2s = _super_pair(nc, pools, rts, w["ew1"], w["ew2"], w["eb1"], w["eb2"],
                          EDGE_H1E, H2E)
        for i in (0, 1):
            p = 2 * q + i
            e0 = p * PN
            # L3 of the two slices in a pair share one PSUM bank via column
            # tiling (partitions 0:64 / 64:128), single accumulation group,
            # k-outer so the two column-group matmuls can run concurrently.
            # Allocated from the l2 tag: its slot-reuse WAR coincides with the
            # natural h2 dependency.
            l3 = psB.tile([128, TN], F32, tag="l2")
            # k-outer so each column-group's LDWEIGHTS overlaps the other
            # group's matmul. start=True on BOTH k0 matmuls: the has_written
            # clear is partition-scoped, so each clears only its own 64
            # partitions (whole-bank clearing would break this ordering).
            for k in (0, 1):
                for t in (0, 1):
                    nc.tensor.matmul(l3[64 * t:64 * (t + 1), :], w["ew3"][:, k],
                                     h2s[i][:, k, t],
                                     start=(k == 0), stop=(k == 1),
                                     skip_group_check=True)
            ot = oot.tile([128, TN], BF16, tag="ot")
            _drain(nc, "A" if p % 2 else "V", ot, l3, w["eb3x"], relu=False)
            nc.sync.dma_start(outT_d[:, e0:e0 + TN], ot[0:64])
            nc.sync.dma_start(outT_d[:, e0 + TN:e0 + PN], ot[64:128])


def _build(mode):
    """mode: 'fused' (node+edge, fe on-chip), 'node', 'edge'."""
    nc = bacc.Bacc("TRN2", target_bir_lowering=False)
    td = _declare_weights(nc)
    if mode in ("fused", "node"):
        xT_d = nc.dram_tensor("xT", [128, RC], BF16, kind="ExternalInput")
    if mode in ("fused", "edge"):
        attrT_d = nc.dram_tensor("attrT", [64, EC], BF16, kind="ExternalInput")
        outT_d = nc.dram_tensor("outT", [64, EC], BF16, kind="ExternalOutput")
    if mode == "edge":
        feTg_d = nc.dram_tensor("feTg", [64, EC], BF16, kind="ExternalInput")
    if mode == "node":
        feT_out = nc.dram_tensor("feT", [ODE, GC], F32, kind="ExternalOutput")

    with tile.TileContext(nc) as tc:
        with (
            tc.tile_pool(name="consts", bufs=1) as consts,
            tc.tile_pool(name="xin", bufs=12) as xin,
            tc.tile_pool(name="hid", bufs=6) as hid,
            tc.tile_pool(name="oot", bufs=8) as oot,
            tc.tile_pool(name="psA", bufs=4, space="PSUM") as psA,
            tc.tile_pool(name="psB", bufs=2, space="PSUM") as psB,
        ):
            pools = (consts, xin, hid, oot, psA, psB)
            # issue the first super-pairs' input DMAs before the 12 weight
            # DMAs so the first matmul's inputs aren't serialized behind
            # them on the SP dispatch queue
            pre_rts = {}
            if mode in ("fused", "node"):
                pre_rts = {q: [_xt_load(nc, pools, xT_d, 2 * q + i)
                               for i in (0, 1)] for q in (0, 1)}
            w = _load_weights(nc, consts, td,
                              node=mode in ("fused", "node"),
                              edge=mode in ("fused", "edge"))
            NQN = RC // (2 * PN)   # node super-pairs (8); 64 graphs each
            NQE = EC // (2 * PN)   # edge super-pairs (32); 16 graphs each
            if mode == "node":
                hsum = consts.tile([128, 2, GC], BF16, tag="hsum")
            if mode == "fused":
                feT_raw = consts.tile([ODE, GC], F32, tag="feT_raw")
                feT_sb = consts.tile([ODE, GC], BF16, tag="feT")
                for q in range(NQN):
                    _node_sp_fe(nc, pools, w, xT_d, feT_raw, q,
                                rts=pre_rts.get(q))
                    if q == NQN // 2 - 1 or q == NQN - 1:
                        # bias+cast feature_enc in halves so the first half
                        # of edge broadcasts unlocks before the node stage
                        # fully finishes (subtile deps)
                        h0 = 0 if q == NQN // 2 - 1 else GC // 2
                        nc.scalar.activation(
                            feT_sb[:, h0:h0 + GC // 2],
                            feT_raw[:, h0:h0 + GC // 2], IDENT, bias=w["nb3"])
                for q in range(NQE):
                    _edge_sp(nc, pools, w, attrT_d, outT_d, ("sbuf", feT_sb), q)
            elif mode == "node":
                for q in range(NQN):
                    _node_sp(nc, pools, w, xT_d, hsum, q, rts=pre_rts.get(q))
                ps_f = psB.tile([ODE, TN], F32, tag="l2")
                for k in (0, 1):
                    nc.tensor.matmul(ps_f, w["nw3"][:, k], hsum[:, k],
                                     start=(k == 0), stop=(k == 1))
                feT_sb = consts.tile([ODE, GC], F32, tag="feT")
                nc.scalar.activation(feT_sb, ps_f, IDENT, bias=w["nb3"])
                nc.sync.dma_start(feT_out[:], feT_sb)
            elif mode == "edge":
                for q in range(NQE):
                    _edge_sp(nc, pools, w, attrT_d, outT_d, ("dram", feTg_d), q)
    nc.finalize()
    return nc


def _get_program(mode):
    if mode not in _PROGRAMS:
        _PROGRAMS[mode] = _build(mode)
    return _PROGRAMS[mode]


def _shared_weight_arrays(kw):
    f = np.float32
    c = np.ascontiguousarray
    eb3 = np.asarray(kw["edge_b3"], dtype=f)
    return {
        "nw1": c(np.asarray(kw["node_w1"], dtype=f).astype(BF16NP)),
        "nw2": c(np.asarray(kw["node_w2"], dtype=f).reshape(2, 128, H).transpose(1, 0, 2).astype(BF16NP)),
        "nw3": c(np.asarray(kw["node_w3"], dtype=f).reshape(2, 128, ODE).transpose(1, 0, 2).astype(BF16NP)),
        "nb1": c(np.asarray(kw["node_b1"], dtype=f).reshape(2, 128).T),
        "nb2": c(np.asarray(kw["node_b2"], dtype=f).reshape(2, 128).T),
        "nb3": c(np.asarray(kw["node_b3"], dtype=f).reshape(ODE, 1)),
        "ew1": c(np.asarray(kw["edge_w1"], dtype=f).astype(BF16NP)),
        "ew2": c(np.asarray(kw["edge_w2"], dtype=f).reshape(2, 128, H).transpose(1, 0, 2).astype(BF16NP)),
        "ew3": c(np.asarray(kw["edge_w3"], dtype=f).reshape(2, 128, ODE).transpose(1, 0, 2).astype(BF16NP)),
        "eb1": c(np.asarray(kw["edge_b1"], dtype=f).reshape(2, 128).T),
        "eb2": c(np.asarray(kw["edge_b2"], dtype=f).reshape(2, 128).T),
        "eb3x": c(np.concatenate([eb3, eb3]).reshape(128, 1)),
    }


def _x_transposed_per_core(x, c):
    xs = np.asarray(x, dtype=np.float32).reshape(G, ODE, 2, NDATA)[c * GC:(c + 1) * GC]
    return np.ascontiguousarray(xs.transpose(1, 2, 0, 3).reshape(128, RC).astype(BF16NP))


def kernel(x, edge_attr, node_w1, node_b1, node_w2, node_b2, node_w3, node_b3,
           edge_w1, edge_b1, edge_w2, edge_b2, edge_w3, edge_b3,
           edge_index, batch):
    global last_results
    kw = dict(x=x, node_w1=node_w1, node_b1=node_b1, node_w2=node_w2,
              node_b2=node_b2, node_w3=node_w3, node_b3=node_b3,
              edge_w1=edge_w1, edge_b1=edge_b1, edge_w2=edge_w2,
              edge_b2=edge_b2, edge_w3=edge_w3, edge_b3=edge_b3)
    trace = os.environ.get("KERNEL_TRACE", "") == "1"
    if trace:
        _install_trace_shim()

    edge_attr = np.asarray(edge_attr, dtype=np.float32)
    ei = np.asarray(edge_index)
    bt = np.asarray(batch)
    g_src = bt[ei[0]]
    g_dst = bt[ei[1]]
    same = g_src == g_dst
    structured = bool((g_src == np.repeat(np.arange(G), EPG)).all())

    shared = _shared_weight_arrays(kw)
    run_kwargs = dict(core_ids=list(range(NCORES)), trace=trace,
                      trace_cores=[0] if trace else None)

    def _attr_perm(c):
        sl = edge_attr[c * EC:(c + 1) * EC][_ECOL]
        return np.ascontiguousarray(sl.T.astype(BF16NP))

    def _out_unperm(outT):
        o = np.empty((EC, EA), dtype=np.float32)
        o[_ECOL] = np.asarray(outT, dtype=np.float32).T
        return o

    if structured:
        nc = _get_program("fused")
        in_maps = []
        for c in range(NCORES):
            m = dict(shared)
            m["xT"] = _x_transposed_per_core(x, c)
            m["attrT"] = _attr_perm(c)
            in_maps.append(m)
        res = run_bass_kernel_spmd(nc, in_maps, **run_kwargs)
        last_results = res
        out = np.empty((E, EA), dtype=np.float32)
        for c in range(NCORES):
            out[c * EC:(c + 1) * EC] = _out_unperm(res.results[c]["outT"])
    else:
        # general path: node stage -> host gather of feature_enc -> edge stage
        nc_node = _get_program("node")
        in_maps = []
        for c in range(NCORES):
            m = dict(shared)
            m["xT"] = _x_transposed_per_core(x, c)
            in_maps.append(m)
        res_n = run_bass_kernel_spmd(nc_node, in_maps, **run_kwargs)
        feT_full = np.concatenate([np.asarray(res_n.results[c]["feT"], dtype=np.float32)
                                   for c in range(NCORES)], axis=1)  # [64, G]
        nc_edge = _get_program("edge")
        in_maps = []
        for c in range(NCORES):
            m = dict(shared)
            m["attrT"] = _attr_perm(c)
            gs = g_src[c * EC:(c + 1) * EC][_ECOL]
            m["feTg"] = np.ascontiguousarray(feT_full[:, gs].astype(BF16NP))
            in_maps.append(m)
        res = run_bass_kernel_spmd(nc_edge, in_maps, **run_kwargs)
        last_results = res
        out = np.empty((E, EA), dtype=np.float32)
        for c in range(NCORES):
            out[c * EC:(c + 1) * EC] = _out_unperm(res.results[c]["outT"])

    if not same.all():
        out = np.where(same[:, None], out, edge_attr)
    return out
